# revision 1
# baseline (speedup 1.0000x reference)
# Trainium2 Bass kernel for BasedCrossAttention (sparse_attention).
#
# Sharding: 8 cores = 2 batches x 4 head-groups (4 heads each).
# Each core computes, for its (batch, 4 heads):
#   rmsnorm(x) -> q / window-kv projections, encoder -> kv projections,
#   Taylor linear cross-attention (redundant F=273 feature map), sliding
#   window (64) causal self-attention, and a partial out-projection.
# Host: transposes inputs once, slices weights per core, sums the 4
# partial out-projections per batch and adds the residual.
#
# On-chip layout is "transposed" (feature-major): activations live as
# [d, t] with d on partitions so every matmul contracts over partitions.
# The attention/out-proj phase (B2) runs channel-major: linear and
# window outputs are produced as [ch, tok] tiles (v / kv_state used as
# the stationary operand) so no PE transposes are needed and every
# moving operand is >=128 wide, keeping the PE dense and HAM-warm.
import math
from contextlib import ExitStack

import ml_dtypes
import numpy as np

import concourse.bass as bass
import concourse.tile as tile
from concourse import bacc, mybir
from concourse.bass_utils import run_bass_kernel_spmd

F32 = mybir.dt.float32
BF = mybir.dt.bfloat16
AF = mybir.ActivationFunctionType

D = 1024
NH = 16
HD = 64
FI = 16  # feature input dim
NQ = 136  # triu quadratic features
FTOT = NQ + FI + 1  # 153: [quad(136), lin(16), ones(1)]
C1 = FTOT - 128  # 25: second F chunk
WIN = 64
EPS_NORM = 1e-6
EPS_DEN = 1e-6
HPC = 4  # heads per core
DC = D // 128  # 8 d-model chunks


def build_program(T=2048, debug=False, dump=False):
    """One SPMD program; per-core variation comes only through inputs."""
    NB = T // 128  # 128-token blocks
    NI4 = T // 512  # 512-token chunks
    assert T % 512 == 0

    nc = bacc.Bacc("TRN2", target_bir_lowering=False, debug=debug, num_devices=8)

    # ---- DRAM I/O ----
    xT = nc.dram_tensor("xT", [D, T], BF, kind="ExternalInput")
    encT = nc.dram_tensor("encT", [D, T], BF, kind="ExternalInput")
    Wq = nc.dram_tensor("Wq", [D, HPC * HD], BF, kind="ExternalInput")
    Wk = nc.dram_tensor("Wk", [D, HPC * HD], BF, kind="ExternalInput")
    Wv = nc.dram_tensor("Wv", [D, HPC * HD], BF, kind="ExternalInput")
    Wwk = nc.dram_tensor("Wwk", [D, HPC * HD], BF, kind="ExternalInput")
    Wwv = nc.dram_tensor("Wwv", [D, HPC * HD], BF, kind="ExternalInput")
    WqfA0 = nc.dram_tensor("WqfA0", [HD, 128], BF, kind="ExternalInput")
    WqfA1 = nc.dram_tensor("WqfA1", [HD, C1 - 1], BF, kind="ExternalInput")
    WqfB0 = nc.dram_tensor("WqfB0", [HD, 128], BF, kind="ExternalInput")
    WqfB1 = nc.dram_tensor("WqfB1", [HD, 8], BF, kind="ExternalInput")
    WkfAB = nc.dram_tensor("WkfAB", [HD, NQ + FI + NQ], BF, kind="ExternalInput")
    WoutA = nc.dram_tensor("WoutA", [HPC * 128, D], BF, kind="ExternalInput")
    maskb_d = nc.dram_tensor("maskb", [128, 512], BF, kind="ExternalInput")
    ident_d = nc.dram_tensor("ident", [128, 128], BF, kind="ExternalInput")
    out_d = nc.dram_tensor("out", [D, T], BF, kind="ExternalOutput")
    if dump:
        dcomb_d = [nc.dram_tensor(f"dcomb{h}", [128, T], BF, kind="ExternalOutput")
                   for h in range(HPC)]
        dkvs_d = [nc.dram_tensor(f"dkvs{h}", [128, 130], BF, kind="ExternalOutput")
                  for h in range(HPC)]
        dq_d = nc.dram_tensor("dq", [128, T], BF, kind="ExternalOutput")
        dkw_d = nc.dram_tensor("dkw", [128, T], BF, kind="ExternalOutput")
        dvwa_d = nc.dram_tensor("dvwa", [128, HPC * NB * 65], BF, kind="ExternalOutput")
        dex_d = nc.dram_tensor("dex", [128, 512], BF, kind="ExternalOutput")
        dphi_d = nc.dram_tensor("dphi", [128, 512], BF, kind="ExternalOutput")
        dwin_d = nc.dram_tensor("dwin", [65, 512], F32, kind="ExternalOutput")
        drb_d = nc.dram_tensor("drb", [128, 512], F32, kind="ExternalOutput")

    with tile.TileContext(nc) as tc, ExitStack() as ctx:
        persist = ctx.enter_context(tc.tile_pool(name="persist", bufs=1))

        def load_w(dram, shape, rearr=None):
            t = persist.tile(shape, BF, name=f"w_{dram.name}", tag=f"w_{dram.name}")
            src = dram.ap() if rearr is None else dram.ap().rearrange(rearr, p=128)
            nc.sync.dma_start(out=t, in_=src)
            return t

        def load_w2(dram, n):
            # small [64, n] weight duplicated into both partition halves so it
            # can pair with operands at base_partition 0 or 64
            t = persist.tile([128, n], BF, name=f"w2_{dram.name}", tag=f"w2_{dram.name}")
            nc.sync.dma_start(out=t[0:64, :], in_=dram.ap())
            nc.sync.dma_start(out=t[64:128, :], in_=dram.ap())
            return t

        # Only the weights phase A1 needs, so the encoder-chunk DMAs start
        # as early as possible; everything else loads later.
        wk = load_w(Wk, [128, DC, HPC * HD], "(c p) n -> p c n")
        wv = load_w(Wv, [128, DC, HPC * HD], "(c p) n -> p c n")

        ones_b = persist.tile([128, 1], BF)
        nc.vector.memset(ones_b, 1.0)
        eps_t = persist.tile([1, 1], F32)
        nc.vector.memset(eps_t, EPS_NORM)

        # Long-lived activations
        kvs = [persist.tile([128, 256], BF, tag=f"kvs{h}", name=f"kvs{h}") for h in range(HPC)]
        kwT = [persist.tile([128, T], BF, tag=f"kwT{hp}", name=f"kwT{hp}") for hp in range(2)]
        qT = [persist.tile([128, T], BF, tag=f"qT{hp}", name=f"qT{hp}") for hp in range(2)]
        vwa = persist.tile([128, HPC, NB, 128], BF, tag="vwa", name="vwa")
        rrT = persist.tile([128, NI4 * 4], F32, tag="rrT", name="rrT")  # rstd token-major

        # =============== Phase A: encoder side -> kv_state ===============
        with tc.tile_pool(name="pAkeep", bufs=1) as pAkeep:
            kT = [pAkeep.tile([128, T], BF, tag=f"kT{hp}", name=f"kT{hp}") for hp in range(2)]
            vA = pAkeep.tile([128, HPC, NB, 128], BF, tag="vA", name="vA")
            nc.vector.memset(vA[:, :, :, 0:1], 1.0)

            encT_r = encT.ap().rearrange("(c p) t -> p c t", p=128)
            ctxA1 = ExitStack()
            ctxA1.enter_context(nc.named_scope("A1_kv_proj"))
            pA = ctxA1.enter_context(tc.tile_pool(name="pA", bufs=2))
            pAps = ctxA1.enter_context(tc.tile_pool(name="pAps1", bufs=2, space="PSUM"))
            # warm the PE clock (HAM) while input DMAs land
            wsc = pA.tile([128, 512], BF, tag="wsc", bufs=1)
            nc.vector.memset(wsc, 0.125)
            wps = pAps.tile([128, 512], F32, tag="warm", bufs=1)
            for _ in range(16):
                nc.tensor.matmul(wps, (wsc[:, 0:128]), (wsc), start=True, stop=True)
            for i4 in range(NI4):
                et = pA.tile([128, DC, 512], BF, tag="enc", bufs=3)
                nc.sync.dma_start(out=et, in_=encT_r[:, :, i4 * 512:(i4 + 1) * 512])
                for hp in range(2):
                    ps = pAps.tile([128, 512], F32, tag="kt", bufs=3)
                    for c in range(DC):
                        nc.tensor.matmul(
                            ps, (wk[:, c, hp * 128:(hp + 1) * 128]),
                            (et[:, c, :]), start=(c == 0), stop=(c == DC - 1))
                    nc.scalar.copy(kT[hp][:, i4 * 512:(i4 + 1) * 512], ps)
                for tb in range(4):
                    blk = i4 * 4 + tb
                    ps = pAps.tile([128, HPC * HD], F32, tag="v")
                    for c in range(DC):
                        nc.tensor.matmul(
                            ps, (et[:, c, tb * 128:(tb + 1) * 128]),
                            (wv[:, c, :]), start=(c == 0), stop=(c == DC - 1))
                    # strided store into per-head 65-wide blocks (col 64 stays 1)
                    if blk % 2 == 0:
                        nc.vector.tensor_copy(vA[:, :, blk, 64:128], ps)
                    else:
                        nc.scalar.copy(vA[:, :, blk, 64:128], ps)

            ctxA1.close()
            with tc.tile_pool(name="pB1", bufs=2) as pB1, \
                 tc.tile_pool(name="pB1ps", bufs=2, space="PSUM") as pB1ps, \
                 nc.named_scope("B1_proj"):
                nc.vector.memset(vwa[:, :, :, 0:1], 1.0)
                xT_r = xT.ap().rearrange("(c p) t -> p c t", p=128)
                wq = load_w(Wq, [128, DC, HPC * HD], "(c p) n -> p c n")
                wwk = load_w(Wwk, [128, DC, HPC * HD], "(c p) n -> p c n")
                wwv = load_w(Wwv, [128, DC, HPC * HD], "(c p) n -> p c n")
                for i4 in range(NI4):
                    tsl = slice(i4 * 512, (i4 + 1) * 512)
                    xt = pB1.tile([128, DC, 512], BF, tag="xt", bufs=3)
                    nc.sync.dma_start(out=xt, in_=xT_r[:, :, tsl])
                    # rmsnorm stats: sumsq over d via ones-matmul
                    ssp = pB1ps.tile([1, 512], F32, tag="ss", bufs=2)
                    for c in range(DC):
                        sq = pB1.tile([128, 512], BF, tag="sq")
                        if c % 2 == 0:
                            nc.scalar.square(sq, xt[:, c, :])
                        else:
                            nc.vector.tensor_mul(sq, xt[:, c, :], xt[:, c, :])
                        nc.tensor.matmul(ssp, ones_b, sq,
                                         start=(c == 0), stop=(c == DC - 1))
                    sd = pB1.tile([1, 512], F32, tag="sd")
                    nc.scalar.activation(sd, ssp, AF.Sqrt, bias=eps_t[0:1, 0:1], scale=1.0 / D)
                    rr = pB1.tile([1, 512], F32, tag="rr")
                    nc.vector.reciprocal_approx_fast(rr, sd)
                    rstdB = pB1.tile([128, 512], F32, tag="rstdB")
                    nc.gpsimd.partition_broadcast(rstdB, rr)
                    # token-major rstd (for vwin scaling): transpose via 4 thin
                    # bf16 matmuls (rr row chunks as stationary, ones as moving)
                    rrb = pB1.tile([1, 512], BF, tag="rrb")
                    nc.scalar.copy(rrb, rr)
                    rtp4 = pB1ps.tile([128, 4], F32, tag="rt", bufs=1)
                    for tb in range(4):
                        nc.tensor.matmul(rtp4[:, tb:tb + 1],
                                         rrb[0:1, tb * 128:(tb + 1) * 128],
                                         ones_b[0:1, 0:1])
                    nc.vector.tensor_copy(rrT[:, i4 * 4:(i4 + 1) * 4], rtp4)
                    # q / kwin projections (column-scaled by rstd)
                    for w_sb, dst in ((wq, qT), (wwk, kwT)):
                        for hp in range(2):
                            ps = pB1ps.tile([128, 512], F32, tag="qk", bufs=3)
                            for c in range(DC):
                                nc.tensor.matmul(
                                    ps, (w_sb[:, c, hp * 128:(hp + 1) * 128]),
                                    (xt[:, c, :]), start=(c == 0), stop=(c == DC - 1))
                            nc.vector.scalar_tensor_tensor(
                                dst[hp][:, tsl], ps, 1.0, rstdB,
                                op0=mybir.AluOpType.mult, op1=mybir.AluOpType.mult)
                    # vwin projection (row/token-scaled by rstd)
                    for tb in range(4):
                        blk = i4 * 4 + tb
                        ps = pB1ps.tile([128, HPC * HD], F32, tag="vw", bufs=2)
                        for c in range(DC):
                            nc.tensor.matmul(
                                ps, (xt[:, c, tb * 128:(tb + 1) * 128]),
                                (wwv[:, c, :]), start=(c == 0), stop=(c == DC - 1))
                        nc.scalar.activation(vwa[:, :, blk, 64:128], ps, AF.Copy,
                                             scale=rrT[:, blk:blk + 1])

            # A2: features + kv_state^T per head (one wide matmul per block)
            wkfAB = load_w2(WkfAB, NQ + FI + NQ)
            ident = load_w(ident_d, [128, 128])
            ctxA2 = ExitStack()
            ctxA2.enter_context(nc.named_scope("A2_kvstate"))
            pA2sb = ctxA2.enter_context(tc.tile_pool(name="pA2sb", bufs=1))
            pAps = ctxA2.enter_context(tc.tile_pool(name="pAps2", bufs=1, space="PSUM"))
            for hp in range(2):
                kvt2 = [pAps.tile([128, FTOT], F32, tag=f"kvt{u}", bufs=1,
                                  name=f"kvt{u}") for u in range(2)]
                for tb in range(NB):
                    ts_ = slice(tb * 128, (tb + 1) * 128)
                    reps = []
                    # paired K=64 matmuls: heads 2hp (rows 0:64), 2hp+1 (64:128)
                    for u in range(2):
                        ho = u * 64
                        rep = pAps.tile([128, NQ + FI + NQ], F32, tag=f"rep{u}",
                                        bufs=1, name=f"rep{u}")
                        nc.tensor.matmul(rep[:, 0:NQ + FI], (kT[hp][ho:ho + 64, ts_]),
                                         (wkfAB[ho:ho + 64, 0:NQ + FI]))
                        nc.tensor.matmul(rep[:, NQ + FI:NQ + FI + NQ],
                                         (kT[hp][ho:ho + 64, ts_]),
                                         (wkfAB[ho:ho + 64, NQ + FI:NQ + FI + NQ]))
                        reps.append(rep)
                    phik2 = []
                    for u in range(2):
                        phik = pA2sb.tile([128, FTOT], BF, tag=f"phik{u}", bufs=2,
                                          name=f"phik{u}")
                        nc.vector.memset(phik[:, NQ + FI:FTOT], 1.0)  # ones col
                        if u == 0:
                            nc.scalar.copy(phik[:, 0:NQ + FI], reps[u][:, 0:NQ + FI])
                        else:
                            nc.vector.tensor_copy(phik[:, 0:NQ + FI],
                                                  reps[u][:, 0:NQ + FI])
                        nc.vector.tensor_mul(phik[:, 0:NQ], phik[:, 0:NQ],
                                             reps[u][:, NQ + FI:NQ + FI + NQ])
                        phik2.append(phik)
                    for u in range(2):
                        nc.tensor.matmul(kvt2[u], (vA[:, 2 * hp + u, tb, :]),
                                         (phik2[u]),
                                         start=(tb == 0), stop=(tb == NB - 1))
                for u in range(2):
                    h = 2 * hp + u
                    # kv_state^T [65, 273] -> F-major kvs[h] via PE transposes
                    kvt_sb = pA2sb.tile([128, FTOT], BF, tag="kvt_sb", bufs=2)
                    nc.vector.tensor_copy(kvt_sb, kvt2[u])
                    tp0 = pAps.tile([128, 128], BF, tag="tp", bufs=2)
                    nc.tensor.transpose(tp0, kvt_sb[:, 0:128], ident)
                    nc.vector.tensor_copy(kvs[h][:, 0:128], tp0)
                    tp1 = pAps.tile([C1, 128], BF, tag="tp", bufs=2)
                    nc.tensor.transpose(tp1, kvt_sb[:, 128:FTOT], ident)
                    nc.vector.tensor_copy(kvs[h][0:C1, 128:256], tp1)
            ctxA2.close()

        # ================= Phase B2: features + attention + out-proj =====
        # Channel-major per 256-token superblock: lin/win computed as
        # [ch, tok] (kv_state / v-blocks stationary) -> no PE transposes.
        # PSUM banks are paired by column-splitting (p0|pb0, p1|pb1,
        # lin|win); out-proj runs 512-wide on every other chunk and shares
        # the score tag, so all 8 banks stay double-buffered.
        wqfA0 = load_w2(WqfA0, 128)
        wqfA1 = load_w2(WqfA1, C1 - 1)
        wqfB0 = load_w2(WqfB0, 128)
        wqfB1 = load_w2(WqfB1, 8)
        maskb = load_w(maskb_d, [128, 512])
        wout = load_w(WoutA, [128, HPC, D], "(h p) n -> p h n")
        NSB = T // 256
        combs_hold = {}
        with tc.tile_pool(name="pB2", bufs=2) as pB2, \
             tc.tile_pool(name="pB2ps", bufs=1, space="PSUM") as pB2ps:
            for j in range(NSB):
                half = j % 2
                co = half * 256
                tsl = slice(j * 256, (j + 1) * 256)
                qsl = tsl
                qslA = slice(j * 256, j * 256 + 128)
                qslB = slice(j * 256 + 128, (j + 1) * 256)

                # ---- window scores + exp (independent of kvs) ----
                exs = {}
                for hp in range(2):
                    # packed scores [kbL q0:128 | kb0 q0:256 | kb1 q128:256],
                    # paired heads in disjoint PE row groups
                    sps = [pB2ps.tile([128, 512], F32, tag="S", bufs=2,
                                      name=f"S{u}") for u in range(2)]
                    # causal mask folded in as a -240 bias via one identity
                    # matmul per score tile (no DVE mask multiply needed)
                    for u in range(2):
                        bsl = slice(0, 512) if j > 0 else slice(128, 512)
                        nc.tensor.matmul(sps[u][:, bsl], (ident),
                                         (maskb[:, bsl]), start=True, stop=False)
                    if j > 0:
                        for u in range(2):
                            ho = u * 64
                            nc.tensor.matmul(
                                sps[u][:, 0:128],
                                (kwT[hp][ho:ho + 64, (2 * j - 1) * 128:2 * j * 128]),
                                (qT[hp][ho:ho + 64, qslA]), start=False, stop=True)
                    for u in range(2):
                        ho = u * 64
                        nc.tensor.matmul(
                            sps[u][:, 128:384],
                            (kwT[hp][ho:ho + 64, 2 * j * 128:(2 * j + 1) * 128]),
                            (qT[hp][ho:ho + 64, qsl]), start=False, stop=True)
                    for u in range(2):
                        ho = u * 64
                        nc.tensor.matmul(
                            sps[u][:, 384:512],
                            (kwT[hp][ho:ho + 64, (2 * j + 1) * 128:(2 * j + 2) * 128]),
                            (qT[hp][ho:ho + 64, qslB]), start=False, stop=True)
                    for u in range(2):
                        ex = pB2.tile([128, 512], BF, tag=f"exps{hp}{u}", bufs=2,
                                      name=f"exps{u}")
                        if j > 0:
                            nc.scalar.activation(ex, sps[u], AF.Exp, scale=0.125)
                        else:
                            nc.scalar.activation(ex[:, 128:512], sps[u][:, 128:512],
                                                 AF.Exp, scale=0.125)
                        exs[(hp, u)] = ex

                # ---- per head: phi_q + linear + window AV + normalize ----
                for h in range(HPC):
                    hp, ho, u = h // 2, (h % 2) * 64, h % 2
                    qtt = qT[hp][ho:ho + 64, tsl]
                    pa = pB2ps.tile([128, 512], F32, tag="pA", bufs=2, name="pa")
                    pbk = pB2ps.tile([40, 512], F32, tag="pB", bufs=1, name="pbk")
                    p0, pb0 = pa[:, 0:256], pa[:, 256:512]
                    p1, pb1 = pbk[0:C1 - 1, 0:256], pbk[32:40, 256:512]
                    nc.tensor.matmul(p0, (wqfA0[ho:ho + 64, :]), (qtt))
                    nc.tensor.matmul(p1, (wqfA1[ho:ho + 64, :]), (qtt))
                    nc.tensor.matmul(pb0, (wqfB0[ho:ho + 64, :]), (qtt))
                    nc.tensor.matmul(pb1, (wqfB1[ho:ho + 64, :]), (qtt))
                    pb_sb = pB2.tile([128, 256], BF, tag="pb_sb", bufs=3)
                    nc.scalar.copy(pb_sb, pb0)
                    phiq0 = pB2.tile([128, 256], BF, tag="phiq0", bufs=3,
                                     name="phiq0")
                    phiq1 = pB2.tile([C1, 256], BF, tag="phiq1", bufs=3,
                                     name="phiq1")
                    nc.vector.tensor_mul(phiq0, p0, pb_sb)
                    if j * HPC + h < 3:
                        # ones row 24 set once per physical buffer (bufs=3);
                        # later allocations never touch it
                        nc.vector.memset(phiq1, 1.0)
                    nc.scalar.copy(phiq1[0:C1 - 1, :], p1)
                    nc.vector.tensor_mul(phiq1[0:8, :], phiq1[0:8, :], pb1)
                    # linear + window AV share one bank: [128, 0:256 | 256:512]
                    # rows: 0 = normalizer (ones col), 64:128 = channels
                    lw = pB2ps.tile([128, 512], F32, tag="lw", bufs=2, name="lw")
                    lin_cm, win_cm = lw[:, 0:256], lw[:, 256:512]
                    nc.tensor.matmul(lin_cm, (kvs[h][:, 0:128]), (phiq0),
                                     start=True, stop=False)
                    nc.tensor.matmul(lin_cm, (kvs[h][0:C1, 128:256]), (phiq1),
                                     start=False, stop=True)
                    exj = exs[(hp, u)]
                    if j > 0:
                        nc.tensor.matmul(win_cm[:, 0:128],
                                         (vwa[:, h, 2 * j - 1, :]),
                                         (exj[:, 0:128]), start=True, stop=False)
                        nc.tensor.matmul(win_cm[:, 0:128],
                                         (vwa[:, h, 2 * j, :]),
                                         (exj[:, 128:256]), start=False, stop=True)
                    else:
                        nc.tensor.matmul(win_cm[:, 0:128],
                                         (vwa[:, h, 0, :]),
                                         (exj[:, 128:256]), start=True, stop=True)
                    nc.tensor.matmul(win_cm[:, 128:256],
                                     (vwa[:, h, 2 * j, :]),
                                     (exj[:, 256:384]), start=True, stop=False)
                    nc.tensor.matmul(win_cm[:, 128:256],
                                     (vwa[:, h, 2 * j + 1, :]),
                                     (exj[:, 384:512]), start=False, stop=True)
                    # normalizers: psum row 0 (both paths) -> one fast
                    # reciprocal -> one partition broadcast
                    rcp = pB2.tile([1, 512], F32, tag="rcp", bufs=4)
                    nc.vector.reciprocal_approx_fast(rcp, lw[0:1, :])
                    rb = pB2.tile([64, 512], F32, tag="rb", bufs=3)
                    nc.gpsimd.partition_broadcast(rb, rcp)
                    if half == 0:
                        combs_hold[h] = pB2.tile([128, 512], BF, tag=f"comb{h}",
                                                 bufs=2, name=f"comb{h}")
                    comb = combs_hold[h]
                    nc.vector.tensor_mul(comb[0:64, co:co + 256],
                                         lin_cm[64:128, :], rb[:, 0:256])
                    nc.vector.tensor_mul(comb[64:128, co:co + 256],
                                         win_cm[64:128, :], rb[:, 256:512])
                    if dump:
                        nc.sync.dma_start(out=dcomb_d[h].ap()[:, tsl],
                                          in_=comb[:, co:co + 256])

                # ---- out-projection, 512 wide, on odd chunks ----
                if half == 1:
                    tsl2 = slice((j - 1) * 256, (j + 1) * 256)
                    for dc in range(DC):
                        po = pB2ps.tile([128, 512], F32, tag="po", bufs=1,
                                        name="po")
                        for h in range(HPC):
                            nc.tensor.matmul(
                                po, (wout[:, h, dc * 128:(dc + 1) * 128]),
                                (combs_hold[h]),
                                start=(h == 0), stop=(h == HPC - 1))
                        ob = pB2.tile([128, 512], BF, tag="ob", bufs=3)
                        if dc % 2 == 0:
                            nc.scalar.copy(ob, po)
                        else:
                            nc.vector.tensor_copy(ob, po)
                        nc.sync.dma_start(
                            out=out_d.ap()[dc * 128:(dc + 1) * 128, tsl2],
                            in_=ob)
            if dump:
                for h in range(HPC):
                    nc.sync.dma_start(out=dkvs_d[h].ap(), in_=kvs[h])
                nc.sync.dma_start(out=dq_d.ap(), in_=qT[0])
                nc.sync.dma_start(out=dkw_d.ap(), in_=kwT[0])
                nc.sync.dma_start(out=dvwa_d.ap(),
                                  in_=vwa.rearrange("p a b c -> p (a b c)"))
    nc.compile()
    return nc


# ---------------- host side ----------------

def _host_prep(x, encoder_out, norm_w, Wq, Wkv, Wqf, Wkf, Wwin, Wout, T):
    """Build the 8 per-core input maps."""
    nw = norm_w.astype(np.float64)
    WqF = (nw[:, None] * Wq).astype(np.float32)
    WwinF = (nw[:, None] * Wwin).astype(np.float32)
    Wk_all, Wv_all = Wkv[:, :D], Wkv[:, D:]
    Wwk_all, Wwv_all = WwinF[:, :D], WwinF[:, D:]

    ti, tj = np.triu_indices(FI)
    sc = np.where(ti == tj, 0.5, 2.0 ** -0.5).astype(np.float64)
    WqfA_f = (sc * Wqf[:, ti]).astype(np.float32)  # [64, 136]
    WqfB_f = Wqf[:, tj]
    WkfA_f = (sc * Wkf[:, ti]).astype(np.float32)
    WkfB_f = Wkf[:, tj]
    WqfA0 = WqfA_f[:, :128]
    WqfA1 = np.concatenate([WqfA_f[:, 128:], Wqf], axis=1)       # [64, 24]
    WqfB0 = WqfB_f[:, :128]
    WqfB1 = np.ascontiguousarray(WqfB_f[:, 128:])                # [64, 8]
    WkfAB = np.concatenate([WkfA_f, Wkf, WkfB_f], axis=1)       # [64, 288]

    kq, qq = np.arange(128)[:, None], np.arange(256)[None, :]
    mask_mid = ((kq <= qq) & (kq >= qq - WIN)).astype(np.float32)
    qq1 = np.arange(128)[None, :]
    mask_left = (kq >= qq1 + WIN).astype(np.float32)
    # packed S layout: [kbL q''0:128 | kb0 q''0:256 | kb1 q''128:256]
    mask_pack = np.concatenate([mask_left, mask_mid, mask_mid[:, 0:128]], axis=1)
    maskb = np.where(mask_pack > 0, 0.0, -240.0).astype(np.float32)
    ident = np.eye(128, dtype=np.float32)

    in_maps = []
    for c in range(8):
        b, g = c // 4, c % 4
        cols = slice(g * HPC * HD, (g + 1) * HPC * HD)
        WoutA = np.empty((HPC * 128, D), np.float32)
        for h in range(HPC):
            hg = g * HPC + h
            WoutA[h * 128:h * 128 + 64] = Wout[hg * 64:(hg + 1) * 64]
            WoutA[h * 128 + 64:(h + 1) * 128] = Wout[D + hg * 64:D + (hg + 1) * 64]
        bf = ml_dtypes.bfloat16
        in_maps.append({
            "xT": np.ascontiguousarray(x[b, :T].T).astype(bf),
            "encT": np.ascontiguousarray(encoder_out[b, :T].T).astype(bf),
            "Wq": np.ascontiguousarray(WqF[:, cols]).astype(bf),
            "Wk": np.ascontiguousarray(Wk_all[:, cols]).astype(bf),
            "Wv": np.ascontiguousarray(Wv_all[:, cols]).astype(bf),
            "Wwk": np.ascontiguousarray(Wwk_all[:, cols]).astype(bf),
            "Wwv": np.ascontiguousarray(Wwv_all[:, cols]).astype(bf),
            "WqfA0": np.ascontiguousarray(WqfA0).astype(bf),
            "WqfA1": np.ascontiguousarray(WqfA1).astype(bf),
            "WqfB0": np.ascontiguousarray(WqfB0).astype(bf),
            "WqfB1": np.ascontiguousarray(WqfB1).astype(bf),
            "WkfAB": np.ascontiguousarray(WkfAB).astype(bf),
            "WoutA": WoutA.astype(bf),
            "maskb": maskb.astype(bf),
            "ident": ident.astype(bf),
        })
    return in_maps


_BUILD_CACHE = {}


def run_sharded(inputs, T=2048, trace=False):
    if T not in _BUILD_CACHE:
        _BUILD_CACHE[T] = build_program(T=T)
    nc = _BUILD_CACHE[T]
    in_maps = _host_prep(T=T, **inputs)
    res = run_bass_kernel_spmd(nc, in_maps, core_ids=list(range(8)), trace=trace)
    x = inputs["x"]
    B = x.shape[0]
    out = np.array(x[:, :T], np.float32, copy=True)
    for c in range(8):
        out[c // 4] += res.results[c]["out"].astype(np.float32).T
    return out, res


def kernel(**inputs):
    inputs = {k: np.asarray(v, np.float32) for k, v in inputs.items()}
    out, _ = run_sharded(inputs, T=2048, trace=False)
    return out

